# revision 12
# baseline (speedup 1.0000x reference)
"""Causal self-attention (B=2, T=2048, D=2048, H=16, d=128) on 8 TRN2 NeuronCores.

Sharding: head-parallel compute, token-parallel output. Core c owns heads
{2c, 2c+1} for both batches: column-parallel QKV projection, per-head RoPE +
causal attention. The per-head attention outputs are exchanged with one
AllToAll per (batch, head), after which every core holds all 16 heads for its
own 256 rows and computes the full output projection locally. Host
concatenates the 8 contiguous row shards.

v3 schedule notes (changes vs v2 baseline):
  - Head: per-ci weight/x DMAs fan out over three engine queues (sync/vector/
    scalar) with per-ci weight TILES so the first matmul waits on exactly two
    128KB transfers, not the whole 5MB stage-in.
  - wo (out_w) streams in during the attention phase instead of colliding
    with the ts1/ts2 x prefetches.
  - Attention: causal diagonal pairs use a pair-uniform column offset
    q0 = 128*(2p-4ts) (also at ts=0); the mask multiply runs on GpSimd and
    the softmax reciprocal on ACT to keep DVE off the critical path.
  - a2a_out -> SBUF (aoG) loads are issued right after each batch's
    collectives (prefetched under the next compute phase) and split into
    128KB chunks so they spread across DMA rings.
  - Output DMA is split into 64KB chunks alternating between two queues.
  - A short stream of throwaway N=64 matmuls bridges the QKV->attention
    transition so the PE clock-gate (HAM) never sees an idle window.
Matmuls run bf16 (1cyc/row); accumulation fp32 in PSUM.
"""
import math
import numpy as np
import ml_dtypes
from contextlib import ExitStack

import concourse.bass as bass
import concourse.tile as tile
from concourse import bacc, mybir
from concourse.bass_utils import run_bass_kernel_spmd

F32 = mybir.dt.float32
BF16 = mybir.dt.bfloat16
BF16_NP = ml_dtypes.bfloat16
AF = mybir.ActivationFunctionType
ALU = mybir.AluOpType

NC_ = 8           # cores
B, T, D = 2, 2048, 2048
H, HD = 16, 128   # heads, head_dim
HPC = H // NC_    # heads per core = 2
TS = 512          # t-super tile
NTS = T // TS     # 4
NCH = D // 128    # 16 contraction chunks
ROWS = T // NC_   # 256 own token rows per batch
SCALE = 1.0 / math.sqrt(HD)


def _build_program():
    nc = bacc.Bacc("TRN2", target_bir_lowering=False, debug=False, num_devices=NC_)

    xT_d = nc.dram_tensor("xT", [B, D, T], BF16, kind="ExternalInput")
    wqk_d = nc.dram_tensor("wqk", [D, 4 * 128], BF16, kind="ExternalInput")
    wv_d = nc.dram_tensor("wv", [D, 2 * 128], BF16, kind="ExternalInput")
    wo_d = nc.dram_tensor("wo", [D, D], BF16, kind="ExternalInput")
    cos_d = nc.dram_tensor("cosT", [128, T], BF16, kind="ExternalInput")
    sin_d = nc.dram_tensor("sinTs", [128, T], BF16, kind="ExternalInput")
    mask_d = nc.dram_tensor("masks", [4, 128, TS], BF16, kind="ExternalInput")
    ones_d = nc.dram_tensor("ones", [128, 128], BF16, kind="ExternalInput")
    bqk_d = nc.dram_tensor("bqk", [128, 4], F32, kind="ExternalInput")
    bv_d = nc.dram_tensor("bv", [1, 2 * 128], BF16, kind="ExternalInput")
    bo_d = nc.dram_tensor("bo", [1, D], F32, kind="ExternalInput")
    out_d = nc.dram_tensor("out", [B, ROWS, D], F32, kind="ExternalOutput")

    with tile.TileContext(nc) as tc:
        with ExitStack() as ctx:
            consts = ctx.enter_context(tc.tile_pool(name="consts", bufs=1))
            qkv = ctx.enter_context(tc.tile_pool(name="qkv", bufs=1))
            dramp = ctx.enter_context(tc.tile_pool(name="dramp", bufs=1, space="DRAM"))

            wqk_r = wqk_d.ap().rearrange("(c p) e -> p c e", p=128)
            wv_r = wv_d.ap().rearrange("(c p) e -> p c e", p=128)
            wo_r = wo_d.ap().rearrange("(h p) o -> p h o", p=128)

            cos_t = consts.tile([128, T], BF16)
            sin_t = consts.tile([128, T], BF16)
            bqk_t = consts.tile([128, 4], F32)
            ones_b = consts.tile([128, 128], BF16)
            mask_t = consts.tile([128, 4, TS], BF16)
            bv_t = consts.tile([128, 2 * 128], BF16)
            bo_t = consts.tile([128, D], F32)
            wo_t = consts.tile([128, H, D], BF16)

            def load_consts():
                # consts follow the critical (b0, ts0) stage-in on the
                # scalar/vector queues
                for q in range(4):
                    nc.scalar.dma_start(out=cos_t[:, q * TS:(q + 1) * TS],
                                        in_=cos_d.ap()[:, q * TS:(q + 1) * TS])
                    nc.scalar.dma_start(out=sin_t[:, q * TS:(q + 1) * TS],
                                        in_=sin_d.ap()[:, q * TS:(q + 1) * TS])
                nc.scalar.dma_start(out=bqk_t, in_=bqk_d.ap())
                nc.scalar.dma_start(out=ones_b, in_=ones_d.ap())
                nc.gpsimd.dma_start(out=mask_t,
                                    in_=mask_d.ap().rearrange("m p n -> p m n"))
                nc.gpsimd.dma_start(out=bv_t,
                                    in_=bv_d.ap().partition_broadcast(128))
                nc.gpsimd.dma_start(out=bo_t,
                                    in_=bo_d.ap().partition_broadcast(128))

            q_t = {b: qkv.tile([128, HPC, T], BF16, tag=f"q{b}", name=f"q_t{b}")
                   for b in range(B)}
            k_t = {b: qkv.tile([128, HPC, T], BF16, tag=f"k{b}", name=f"k_t{b}")
                   for b in range(B)}
            v_t = {b: qkv.tile([128, NTS * 4, HPC, 128], BF16, tag=f"v{b}",
                               name=f"v_t{b}") for b in range(B)}

            def stage1(b, xt0, new_xt, qep, tmp, s1ps):
                for ts in range(NTS):
                    qkp = [s1ps.tile([128, TS], F32, tag=f"qkp{j}", name=f"qkp{j}")
                           for j in range(4)]
                    vp = [s1ps.tile([128, 2 * 128], F32, tag=f"vp{tb}",
                                    name=f"vp{tb}")[:] for tb in range(4)]
                    for ci in range(NCH):
                        if b == 0 and ts == 0:
                            xt = xt0[ci]
                        else:
                            xt = new_xt()
                            nc.sync.dma_start(
                                out=xt,
                                in_=xT_d.ap()[b, ci * 128:(ci + 1) * 128,
                                              ts * TS:(ts + 1) * TS],
                            )
                        st_, sp_ = ci == 0, ci == NCH - 1
                        for j in range(4):
                            nc.tensor.matmul(
                                qkp[j][:], wqk_t[ci][:, j * 128:(j + 1) * 128],
                                xt[:], start=st_, stop=sp_)
                        for tb in range(4):
                            nc.tensor.matmul(
                                vp[tb], xt[:, tb * 128:(tb + 1) * 128],
                                wv_t[ci][:], start=st_, stop=sp_)
                    # evict q/k to bf16 on ACT (plus a half-swapped copy for
                    # rotate_half); RoPE + bias fused on DVE. sinTs rows 0:64
                    # carry the rotate_half sign flip.
                    cs = cos_t[:, ts * TS:(ts + 1) * TS]
                    sn = sin_t[:, ts * TS:(ts + 1) * TS]
                    last_tile = (b == B - 1 and ts == NTS - 1)
                    if last_tile:
                        for tb in range(4):
                            vdst = v_t[b][:, ts * 4 + tb, :, :]
                            nc.scalar.activation(
                                vdst, vp[tb].rearrange("p (h e) -> p h e", h=HPC),
                                AF.Copy)
                            nc.vector.tensor_add(
                                vdst, vdst,
                                bv_t[:].rearrange("p (h e) -> p h e", h=HPC))
                    for j in range(4):
                        qe = qep.tile([128, TS], BF16, tag=f"qe{j}", name=f"qe{j}",
                                      bufs=3)
                        qs = qep.tile([128, TS], BF16, tag=f"qs{j}", name=f"qs{j}",
                                      bufs=2)
                        nc.scalar.activation(qe[:], qkp[j][:], AF.Copy)
                        nc.scalar.activation(qs[0:64, :], qe[64:128, :], AF.Copy)
                        nc.scalar.activation(qs[64:128, :], qe[0:64, :], AF.Copy)
                        t1 = tmp.tile([128, TS], BF16, tag="t1", bufs=2)
                        t2 = tmp.tile([128, TS], BF16, tag="t2", bufs=2)
                        nc.vector.tensor_mul(t1[:], qe[:], cs)
                        nc.vector.tensor_mul(t2[:], qs[:], sn)
                        dst = (q_t[b] if j < 2 else k_t[b])[:, j % 2,
                                                            ts * TS:(ts + 1) * TS]
                        nc.vector.scalar_tensor_tensor(
                            dst, t1[:], bqk_t[:, j:j + 1], t2[:], ALU.add, ALU.add)
                    if not last_tile:
                        for tb in range(4):
                            vdst = v_t[b][:, ts * 4 + tb, :, :]
                            nc.scalar.activation(
                                vdst, vp[tb].rearrange("p (h e) -> p h e", h=HPC),
                                AF.Copy)
                            nc.vector.tensor_add(
                                vdst, vdst,
                                bv_t[:].rearrange("p (h e) -> p h e", h=HPC))

            def attention(b, atps, prp, accp, bsp, aosp, warm, npairs_done):
                # one AllToAll per (b, head); triggered as soon as that head's
                # normalized outputs are in DRAM. The per-tile epilogue
                # (denominator matmul, reciprocal, normalize, DRAM write) is
                # deferred until the next tile's first score pair so the PE
                # never waits on the ACT/gpsimd accumulation chain.
                a2a_in = [dramp.tile([NC_, 128, ROWS], BF16, tag=f"a2i{b}{hh}",
                                     name=f"a2i{b}{hh}") for hh in range(HPC)]
                a2a_out = [dramp.tile([NC_, 128, ROWS], BF16, tag=f"a2o{b}{hh}",
                                      name=f"a2o{b}{hh}") for hh in range(HPC)]

                def epilogue(pend):
                    op, acc, hh, ts = pend
                    sm = atps.tile([128, TS], F32, tag="sm", bufs=1)
                    nc.tensor.matmul(sm[:], ones_b[:], acc[:], start=True,
                                     stop=True)
                    bsb = bsp.tile([128, TS], F32, tag="bsb", bufs=2)
                    with nc.allow_low_precision(reason="softmax recip"):
                        nc.vector.reciprocal_approx_fast(bsb[:], sm[:])
                    aos = aosp.tile([128, TS], BF16, tag="aos", bufs=4)
                    nc.vector.tensor_mul(aos[:], op[:], bsb[:])
                    nc.gpsimd.dma_start(
                        out=a2a_in[hh][2 * ts:2 * ts + 2, :, :].transpose(
                            [1, 0, 2]),
                        in_=aos[:].rearrange("d (s q) -> d s q", s=2))

                pend = None
                for hh in range(HPC):
                    for ts in range(NTS):
                        op = None
                        acc = accp.tile([128, TS], BF16, tag="acc", bufs=2)
                        npair = 2 * (ts + 1)
                        prev = None
                        for p in range(npair):
                            st = atps.tile([128, 2, TS], F32, tag="st", bufs=2)
                            # diagonal pairs: skip the columns whose queries
                            # sit fully below every key tile of the pair. The
                            # mask multiply zeroes the skipped region.
                            diag = p >= 2 * ts
                            q0 = max(0, 128 * (2 * p - 4 * ts)) if diag else 0
                            for h2 in range(2):
                                tk = 2 * p + h2
                                nc.tensor.matmul(
                                    st[:, h2, q0:],
                                    k_t[b][:, hh, tk * 128:(tk + 1) * 128],
                                    q_t[b][:, hh,
                                          ts * TS + q0:(ts + 1) * TS],
                                    start=True, stop=True)
                            if op is None:
                                op = atps.tile([128, TS], F32, tag="op", bufs=2)
                            if p == 1 and pend is not None:
                                epilogue(pend)
                                pend = None
                            pr = prp.tile([128, 2, TS], BF16, tag="pr", bufs=4)
                            # first 4 pairs: pr buffers are uninitialized
                            # SBUF; exp full-width so no stale bits (possibly
                            # NaN) survive into the masked multiply.
                            qe0 = q0 if npairs_done[0] >= 4 else 0
                            nc.scalar.activation(pr[:, :, qe0:], st[:, :, qe0:],
                                                 AF.Exp, scale=SCALE)
                            npairs_done[0] += 1
                            if diag:  # zero masked scores (and skipped cols)
                                mi = p - 2 * ts
                                nc.gpsimd.tensor_mul(
                                    pr[:], pr[:], mask_t[:, 2 * mi:2 * mi + 2, :])
                            ps = bsp.tile([128, TS], BF16, tag="ps", bufs=3)
                            nc.vector.tensor_add(ps[:], pr[:, 0, :], pr[:, 1, :])
                            if p == 0:
                                nc.vector.tensor_scalar_add(acc[:], ps[:], 0.0)
                            else:
                                nc.vector.tensor_add(acc[:], acc[:], ps[:])
                            if prev is not None:
                                pp, ppr = prev
                                for h2 in range(2):
                                    tkl = 2 * pp + h2 - 4 * ts
                                    a0 = 128 * tkl if tkl > 0 else 0
                                    nc.tensor.matmul(
                                        op[:, a0:], v_t[b][:, 2 * pp + h2, hh, :],
                                        ppr[:, h2, a0:],
                                        start=(pp == 0 and h2 == 0), stop=False)
                            prev = (p, pr)
                        pp, ppr = prev
                        for h2 in range(2):
                            tkl = 2 * pp + h2 - 4 * ts
                            a0 = 128 * tkl if tkl > 0 else 0
                            nc.tensor.matmul(
                                op[:, a0:], v_t[b][:, 2 * pp + h2, hh, :],
                                ppr[:, h2, a0:],
                                start=(pp == 0 and h2 == 0), stop=(h2 == 1))
                        pend = (op, acc, hh, ts)
                    # flush before the collective: it needs every tile's aos
                    epilogue(pend)
                    pend = None
                    nc.gpsimd.collective_compute(
                        "AllToAll", mybir.AluOpType.bypass,
                        replica_groups=[list(range(NC_))],
                        ins=[a2a_in[hh].opt()], outs=[a2a_out[hh].opt()])
                return a2a_out

            def load_aog(b, a2a_out, aogp):
                # prefetch a2a results into SBUF in 128KB chunks right after
                # the collectives are issued; runs under the following phase.
                aoG = [aogp.tile([128, NC_, ROWS], BF16, tag=f"aoG{b}{hh}",
                                 name=f"aoG{b}{hh}") for hh in range(HPC)]
                for hh in range(HPC):
                    src = a2a_out[hh][:, :, :].rearrange("s d q -> d s q")
                    for c in range(4):
                        nc.sync.dma_start(out=aoG[hh][:, 2 * c:2 * c + 2, :],
                                          in_=src[:, 2 * c:2 * c + 2, :])
                return aoG

            def outproj(b, aoG, yop, yps):
                # aoG[hh][d, src, q] == head (2*src+hh) for my ROWS of batch b
                # tile-at-a-time: each (tb, nb) chunk accumulates its 16
                # head contributions back-to-back, then evacuates while the
                # next chunk computes — output writes pipeline instead of
                # bursting at the end.
                for tb in range(2):
                    for nb in range(4):
                        yp = yps.tile([128, TS], F32, tag="yp", bufs=4)
                        for hh in range(HPC):
                            for s in range(NC_):
                                nc.tensor.matmul(
                                    yp[:], aoG[hh][:, s, tb * 128:(tb + 1) * 128],
                                    wo_t[:, 2 * s + hh, nb * TS:(nb + 1) * TS],
                                    start=(hh == 0 and s == 0),
                                    stop=(hh == 1 and s == NC_ - 1))
                        yo = yop.tile([128, TS], F32, tag="yo", bufs=4)
                        nc.vector.tensor_add(yo[:], yp[:],
                                             bo_t[:, nb * TS:(nb + 1) * TS])
                        # all output writes ride the scalar queue: the sync
                        # queue is parked on the next batch's a2a_out waits
                        # and would stall yo buffer recycling.
                        for c in range(4):
                            nc.scalar.dma_start(
                                out=out_d.ap()[b, tb * 128:(tb + 1) * 128,
                                               nb * TS + c * 128:
                                               nb * TS + (c + 1) * 128],
                                in_=yo[:, c * 128:(c + 1) * 128])

            # ---- phase 1: QKV projections for both batches (no collectives)
            with tc.tile_pool(name="s1w", bufs=1) as s1w, \
                    tc.tile_pool(name="xp", bufs=1) as xp, \
                    tc.tile_pool(name="qep", bufs=1) as qep, \
                    tc.tile_pool(name="tmp", bufs=1) as tmp, \
                    tc.tile_pool(name="s1ps", bufs=1, space="PSUM") as s1ps:
                wqk_t = [s1w.tile([128, 4 * 128], BF16, tag=f"wqk{ci}",
                                  name=f"wqk{ci}") for ci in range(NCH)]
                wv_t = [s1w.tile([128, 2 * 128], BF16, tag=f"wv{ci}",
                                 name=f"wv{ci}") for ci in range(NCH)]

                def new_xt():
                    return xp.tile([128, TS], BF16, tag="xt", name="xt", bufs=16)

                # head: the (b0, ts0) stage-in fans out over three queues so
                # the first matmul waits on exactly one weight chunk + one x
                # tile (256KB), not the whole stage-in.
                xt0 = []
                for ci in range(NCH):
                    nc.scalar.dma_start(out=wqk_t[ci], in_=wqk_r[:, ci, :])
                    nc.scalar.dma_start(out=wv_t[ci], in_=wv_r[:, ci, :])
                    xt = new_xt()
                    nc.sync.dma_start(
                        out=xt, in_=xT_d.ap()[0, ci * 128:(ci + 1) * 128, 0:TS])
                    xt0.append(xt)
                load_consts()
                for b in range(B):
                    stage1(b, xt0, new_xt, qep, tmp, s1ps)

            # wo streams in while attention runs (DMA rings are idle there)
            for ci in range(NCH):
                nc.sync.dma_start(out=wo_t[:, ci, :], in_=wo_r[:, ci, :])

            # ---- phase 2: attention + exchanges --------------------------
            a2a_outs = {}
            aoGs = {}
            npairs_done = [0]
            with tc.tile_pool(name="aogp", bufs=1) as aogp:
                with tc.tile_pool(name="atps", bufs=1, space="PSUM") as atps, \
                        tc.tile_pool(name="prp", bufs=1) as prp, \
                        tc.tile_pool(name="accp", bufs=1) as accp, \
                        tc.tile_pool(name="bsp", bufs=1) as bsp, \
                        tc.tile_pool(name="aosp", bufs=1) as aosp:
                    # pre-allocate PSUM tags in bank order; 'warm' lands on
                    # the spare 8th bank, hosting throwaway warm-up matmuls.
                    for _ in range(2):
                        atps.tile([128, 2, TS], F32, tag="st", bufs=2,
                                  name="st")
                        atps.tile([128, TS], F32, tag="op", bufs=2, name="op")
                    atps.tile([128, TS], F32, tag="sm", bufs=1, name="sm")
                    warm = atps.tile([64, 64], F32, tag="warm", bufs=1,
                                     name="warm")
                    # bridge the QKV->attention hand-off: these dummies
                    # depend on the last batch's v eviction, so they execute
                    # exactly in the transition window and keep HAM at full
                    # clock.
                    for _ in range(80):
                        nc.tensor.matmul(warm[:], ones_b[:, 0:64],
                                         v_t[B - 1][:, NTS * 4 - 1, 1, 0:64],
                                         start=True, stop=True)
                    for b in range(B):
                        a2a_outs[b] = attention(b, atps, prp, accp, bsp, aosp,
                                                warm, npairs_done)
                        aoGs[b] = load_aog(b, a2a_outs[b], aogp)

                # ---- phase 3: output projections -------------------------
                with tc.tile_pool(name="yps", bufs=1, space="PSUM") as yps, \
                        tc.tile_pool(name="yop", bufs=1) as yop:
                    for b in range(B):
                        outproj(b, aoGs[b], yop, yps)

    nc.compile()
    return nc


_NC_CACHE = None


def _get_program():
    global _NC_CACHE
    if _NC_CACHE is None:
        _NC_CACHE = _build_program()
    return _NC_CACHE


def make_in_maps(x, rope_cos, rope_sin, qkv_w, qkv_b, out_w, out_b):
    x = np.asarray(x, dtype=np.float32)
    qkv_w = np.asarray(qkv_w, dtype=np.float32)
    qkv_b = np.asarray(qkv_b, dtype=np.float32)
    out_w = np.asarray(out_w, dtype=np.float32)
    out_b = np.asarray(out_b, dtype=np.float32)

    xT = np.ascontiguousarray(x.transpose(0, 2, 1)).astype(BF16_NP)  # [B, D, T]
    cosT = np.ascontiguousarray(np.asarray(rope_cos, np.float32)[0, 0].T).astype(BF16_NP)
    sinTs = np.ascontiguousarray(np.asarray(rope_sin, np.float32)[0, 0].T).copy()
    sinTs[0:64, :] *= -1.0
    sinTs = sinTs.astype(BF16_NP)

    tk_idx = np.arange(128)[:, None]
    tq_idx = np.arange(TS)[None, :]
    masks = np.stack(
        [np.where(mi * 128 + tk_idx <= tq_idx, 1.0, 0.0) for mi in range(4)]
    ).astype(BF16_NP)                                           # [4, 128, TS]
    ones = np.ones((128, 128), BF16_NP)
    wo = np.ascontiguousarray(out_w.T).astype(BF16_NP)          # [D, D]
    bo = out_b.reshape(1, D)

    in_maps = []
    for c in range(NC_):
        h0 = HPC * c
        qr = qkv_w[h0 * 128:(h0 + HPC) * 128]                  # [256, D]
        kr = qkv_w[D + h0 * 128:D + (h0 + HPC) * 128]
        vr = qkv_w[2 * D + h0 * 128:2 * D + (h0 + HPC) * 128]
        wqk = np.ascontiguousarray(np.concatenate([qr, kr], 0).T).astype(BF16_NP)
        wv = np.ascontiguousarray(vr.T).astype(BF16_NP)        # [D, 256]
        bqk = np.stack(
            [qkv_b[h0 * 128:(h0 + 1) * 128],
             qkv_b[(h0 + 1) * 128:(h0 + 2) * 128],
             qkv_b[D + h0 * 128:D + (h0 + 1) * 128],
             qkv_b[D + (h0 + 1) * 128:D + (h0 + 2) * 128]], axis=1)  # [128, 4]
        bv = qkv_b[2 * D + h0 * 128:2 * D + (h0 + HPC) * 128].reshape(1, 256)
        in_maps.append({
            "xT": xT, "wqk": wqk, "wv": wv, "wo": wo,
            "cosT": cosT, "sinTs": sinTs, "masks": masks, "ones": ones,
            "bqk": np.ascontiguousarray(bqk),
            "bv": np.ascontiguousarray(bv).astype(BF16_NP),
            "bo": bo,
        })
    return in_maps


def assemble(results):
    y = np.empty((B, T, D), dtype=np.float32)
    for c in range(NC_):
        y[:, c * ROWS:(c + 1) * ROWS, :] = results[c]["out"]
    return y


def run(inputs, trace=False, trace_cores=None):
    nc = _get_program()
    in_maps = make_in_maps(**inputs)
    res = run_bass_kernel_spmd(
        nc, in_maps, list(range(NC_)), trace=trace,
        trace_cores=trace_cores if trace else None)
    return assemble(res.results), res


def kernel(**inputs) -> np.ndarray:
    y, _ = run(inputs, trace=False)
    return y


# revision 15
# speedup vs baseline: 1.0572x; 1.0572x over previous
"""Causal self-attention (B=2, T=2048, D=2048, H=16, d=128) on 8 TRN2 NeuronCores.

Sharding: head-parallel compute, token-parallel output. Core c owns heads
{2c, 2c+1} for both batches: column-parallel QKV projection, per-head RoPE +
causal attention. The per-head attention outputs are exchanged with one
AllToAll per (batch, head), after which every core holds all 16 heads for its
own 256 rows and computes the full output projection locally. Host
concatenates the 8 contiguous row shards.

v3 schedule notes (changes vs v2 baseline):
  - Head: per-ci weight/x DMAs fan out over three engine queues (sync/vector/
    scalar) with per-ci weight TILES so the first matmul waits on exactly two
    128KB transfers, not the whole 5MB stage-in.
  - wo (out_w) streams in during the attention phase instead of colliding
    with the ts1/ts2 x prefetches.
  - Attention: causal diagonal pairs use a pair-uniform column offset
    q0 = 128*(2p-4ts) (also at ts=0); the mask multiply runs on GpSimd and
    the softmax reciprocal on ACT to keep DVE off the critical path.
  - a2a_out -> SBUF (aoG) loads are issued right after each batch's
    collectives (prefetched under the next compute phase) and split into
    128KB chunks so they spread across DMA rings.
  - Output DMA is split into 64KB chunks alternating between two queues.
  - A short stream of throwaway N=64 matmuls bridges the QKV->attention
    transition so the PE clock-gate (HAM) never sees an idle window.
Matmuls run bf16 (1cyc/row); accumulation fp32 in PSUM.
"""
import math
import numpy as np
import ml_dtypes
from contextlib import ExitStack

import concourse.bass as bass
import concourse.tile as tile
from concourse import bacc, mybir
from concourse.bass_utils import run_bass_kernel_spmd

F32 = mybir.dt.float32
BF16 = mybir.dt.bfloat16
BF16_NP = ml_dtypes.bfloat16
AF = mybir.ActivationFunctionType
ALU = mybir.AluOpType

NC_ = 8           # cores
B, T, D = 2, 2048, 2048
H, HD = 16, 128   # heads, head_dim
HPC = H // NC_    # heads per core = 2
TS = 512          # t-super tile
NTS = T // TS     # 4
NCH = D // 128    # 16 contraction chunks
ROWS = T // NC_   # 256 own token rows per batch
SCALE = 1.0 / math.sqrt(HD)


def _build_program():
    nc = bacc.Bacc("TRN2", target_bir_lowering=False, debug=False, num_devices=NC_)

    xT_d = nc.dram_tensor("xT", [B, D, T], BF16, kind="ExternalInput")
    wqk_d = nc.dram_tensor("wqk", [D, 4 * 128], BF16, kind="ExternalInput")
    wv_d = nc.dram_tensor("wv", [D, 2 * 128], BF16, kind="ExternalInput")
    wo_d = nc.dram_tensor("wo", [D, D], BF16, kind="ExternalInput")
    cos_d = nc.dram_tensor("cosT", [128, T], BF16, kind="ExternalInput")
    sin_d = nc.dram_tensor("sinTs", [128, T], BF16, kind="ExternalInput")
    mask_d = nc.dram_tensor("masks", [4, 128, TS], BF16, kind="ExternalInput")
    ones_d = nc.dram_tensor("ones", [128, 128], BF16, kind="ExternalInput")
    bqk_d = nc.dram_tensor("bqk", [128, 4], F32, kind="ExternalInput")
    bv_d = nc.dram_tensor("bv", [1, 2 * 128], BF16, kind="ExternalInput")
    bo_d = nc.dram_tensor("bo", [1, D], F32, kind="ExternalInput")
    out_d = nc.dram_tensor("out", [B, ROWS, D], F32, kind="ExternalOutput")

    with tile.TileContext(nc) as tc:
        with ExitStack() as ctx:
            consts = ctx.enter_context(tc.tile_pool(name="consts", bufs=1))
            qkv = ctx.enter_context(tc.tile_pool(name="qkv", bufs=1))
            dramp = ctx.enter_context(tc.tile_pool(name="dramp", bufs=1, space="DRAM"))

            wqk_r = wqk_d.ap().rearrange("(c p) e -> p c e", p=128)
            wv_r = wv_d.ap().rearrange("(c p) e -> p c e", p=128)
            wo_r = wo_d.ap().rearrange("(h p) o -> p h o", p=128)

            cos_t = consts.tile([128, T], BF16)
            sin_t = consts.tile([128, T], BF16)
            bqk_t = consts.tile([128, 4], F32)
            ones_b = consts.tile([128, 128], BF16)
            mask_t = consts.tile([128, 4, TS], BF16)
            bv_t = consts.tile([128, 2 * 128], BF16)
            bo_t = consts.tile([128, D], F32)
            wo_t = consts.tile([128, H, D], BF16)

            def load_consts():
                # consts follow the critical (b0, ts0) stage-in on the
                # scalar/vector queues
                for q in range(4):
                    nc.scalar.dma_start(out=cos_t[:, q * TS:(q + 1) * TS],
                                        in_=cos_d.ap()[:, q * TS:(q + 1) * TS])
                    nc.scalar.dma_start(out=sin_t[:, q * TS:(q + 1) * TS],
                                        in_=sin_d.ap()[:, q * TS:(q + 1) * TS])
                nc.scalar.dma_start(out=bqk_t, in_=bqk_d.ap())
                nc.scalar.dma_start(out=ones_b, in_=ones_d.ap())
                nc.gpsimd.dma_start(out=mask_t,
                                    in_=mask_d.ap().rearrange("m p n -> p m n"))
                nc.gpsimd.dma_start(out=bv_t,
                                    in_=bv_d.ap().partition_broadcast(128))
                nc.gpsimd.dma_start(out=bo_t,
                                    in_=bo_d.ap().partition_broadcast(128))

            q_t = {b: qkv.tile([128, HPC, T], BF16, tag=f"q{b}", name=f"q_t{b}")
                   for b in range(B)}
            k_t = {b: qkv.tile([128, HPC, T], BF16, tag=f"k{b}", name=f"k_t{b}")
                   for b in range(B)}
            v_t = {b: qkv.tile([128, NTS * 4, HPC, 128], BF16, tag=f"v{b}",
                               name=f"v_t{b}") for b in range(B)}

            def stage1(b, xt0, new_xt, qep, tmp, s1ps):
                for ts in range(NTS):
                    qkp = [s1ps.tile([128, TS], F32, tag=f"qkp{j}", name=f"qkp{j}")
                           for j in range(4)]
                    vp = [s1ps.tile([128, 2 * 128], F32, tag=f"vp{tb}",
                                    name=f"vp{tb}")[:] for tb in range(4)]
                    for ci in range(NCH):
                        if b == 0 and ts == 0:
                            xt = xt0[ci]
                        else:
                            xt = new_xt()
                            nc.sync.dma_start(
                                out=xt,
                                in_=xT_d.ap()[b, ci * 128:(ci + 1) * 128,
                                              ts * TS:(ts + 1) * TS],
                            )
                        st_, sp_ = ci == 0, ci == NCH - 1
                        for j in range(4):
                            nc.tensor.matmul(
                                qkp[j][:], wqk_t[ci][:, j * 128:(j + 1) * 128],
                                xt[:], start=st_, stop=sp_)
                        for tb in range(4):
                            nc.tensor.matmul(
                                vp[tb], xt[:, tb * 128:(tb + 1) * 128],
                                wv_t[ci][:], start=st_, stop=sp_)
                    # evict q/k to bf16 on ACT (plus a half-swapped copy for
                    # rotate_half); RoPE + bias fused on DVE. sinTs rows 0:64
                    # carry the rotate_half sign flip.
                    cs = cos_t[:, ts * TS:(ts + 1) * TS]
                    sn = sin_t[:, ts * TS:(ts + 1) * TS]
                    last_tile = (b == B - 1 and ts == NTS - 1)
                    if last_tile:
                        for tb in range(4):
                            vdst = v_t[b][:, ts * 4 + tb, :, :]
                            nc.scalar.activation(
                                vdst, vp[tb].rearrange("p (h e) -> p h e", h=HPC),
                                AF.Copy)
                            nc.vector.tensor_add(
                                vdst, vdst,
                                bv_t[:].rearrange("p (h e) -> p h e", h=HPC))
                    for j in range(4):
                        qe = qep.tile([128, TS], BF16, tag=f"qe{j}", name=f"qe{j}",
                                      bufs=3)
                        qs = qep.tile([128, TS], BF16, tag=f"qs{j}", name=f"qs{j}",
                                      bufs=2)
                        nc.scalar.activation(qe[:], qkp[j][:], AF.Copy)
                        nc.scalar.activation(qs[0:64, :], qe[64:128, :], AF.Copy)
                        nc.scalar.activation(qs[64:128, :], qe[0:64, :], AF.Copy)
                        t1 = tmp.tile([128, TS], BF16, tag="t1", bufs=2)
                        t2 = tmp.tile([128, TS], BF16, tag="t2", bufs=2)
                        nc.vector.tensor_mul(t1[:], qe[:], cs)
                        nc.vector.tensor_mul(t2[:], qs[:], sn)
                        dst = (q_t[b] if j < 2 else k_t[b])[:, j % 2,
                                                            ts * TS:(ts + 1) * TS]
                        nc.vector.scalar_tensor_tensor(
                            dst, t1[:], bqk_t[:, j:j + 1], t2[:], ALU.add, ALU.add)
                    if not last_tile:
                        for tb in range(4):
                            vdst = v_t[b][:, ts * 4 + tb, :, :]
                            nc.scalar.activation(
                                vdst, vp[tb].rearrange("p (h e) -> p h e", h=HPC),
                                AF.Copy)
                            nc.vector.tensor_add(
                                vdst, vdst,
                                bv_t[:].rearrange("p (h e) -> p h e", h=HPC))

            def attention(b, atps, prp, accp, bsp, aosp, warm, npairs_done):
                # one AllToAll per (b, head); triggered as soon as that head's
                # normalized outputs are in DRAM. The per-tile epilogue
                # (denominator matmul, reciprocal, normalize, DRAM write) is
                # deferred until the next tile's first score pair so the PE
                # never waits on the ACT/gpsimd accumulation chain.
                a2a_in = [dramp.tile([NC_, 128, ROWS], BF16, tag=f"a2i{b}{hh}",
                                     name=f"a2i{b}{hh}") for hh in range(HPC)]
                a2a_out = [dramp.tile([NC_, 128, ROWS], BF16, tag=f"a2o{b}{hh}",
                                      name=f"a2o{b}{hh}") for hh in range(HPC)]

                def epilogue(pend):
                    op, acc, hh, ts = pend
                    sm = atps.tile([128, TS], F32, tag="sm", bufs=1)
                    nc.tensor.matmul(sm[:], ones_b[:], acc[:], start=True,
                                     stop=True)
                    bsb = bsp.tile([128, TS], F32, tag="bsb", bufs=2)
                    with nc.allow_low_precision(reason="softmax recip"):
                        nc.vector.reciprocal_approx_fast(bsb[:], sm[:])
                    aos = aosp.tile([128, TS], BF16, tag="aos", bufs=4)
                    nc.vector.tensor_mul(aos[:], op[:], bsb[:])
                    nc.gpsimd.dma_start(
                        out=a2a_in[hh][2 * ts:2 * ts + 2, :, :].transpose(
                            [1, 0, 2]),
                        in_=aos[:].rearrange("d (s q) -> d s q", s=2))

                pend = None
                for hh in range(HPC):
                    for ts in range(NTS):
                        op = None
                        acc = accp.tile([128, TS], BF16, tag="acc", bufs=2)
                        npair = 2 * (ts + 1)
                        prev = None
                        for p in range(npair):
                            st = atps.tile([128, 2, TS], F32, tag="st", bufs=2)
                            # diagonal pairs: skip the columns whose queries
                            # sit fully below every key tile of the pair. The
                            # mask multiply zeroes the skipped region.
                            diag = p >= 2 * ts
                            q0 = max(0, 128 * (2 * p - 4 * ts)) if diag else 0
                            for h2 in range(2):
                                tk = 2 * p + h2
                                nc.tensor.matmul(
                                    st[:, h2, q0:],
                                    k_t[b][:, hh, tk * 128:(tk + 1) * 128],
                                    q_t[b][:, hh,
                                          ts * TS + q0:(ts + 1) * TS],
                                    start=True, stop=True)
                            if op is None:
                                op = atps.tile([128, TS], F32, tag="op", bufs=2)
                            if p == 1 and pend is not None:
                                epilogue(pend)
                                pend = None
                            pr = prp.tile([128, 2, TS], BF16, tag="pr", bufs=4)
                            # first 4 pairs: pr buffers are uninitialized
                            # SBUF; exp full-width so no stale bits (possibly
                            # NaN) survive into the masked multiply.
                            qe0 = q0 if npairs_done[0] >= 4 else 0
                            nc.scalar.activation(pr[:, :, qe0:], st[:, :, qe0:],
                                                 AF.Exp, scale=SCALE)
                            npairs_done[0] += 1
                            if diag:  # zero masked scores (and skipped cols)
                                mi = p - 2 * ts
                                nc.vector.tensor_mul(
                                    pr[:], pr[:], mask_t[:, 2 * mi:2 * mi + 2, :])
                            if p == 0:
                                nc.vector.tensor_add(acc[:], pr[:, 0, :],
                                                     pr[:, 1, :])
                            else:
                                # the pair-sum feeds only the (deferred)
                                # denominator, so its latency is slack:
                                # non-diagonal pair-sums run on GpSimd to
                                # keep DVE off the critical path.
                                ps = bsp.tile([128, TS], BF16, tag="ps", bufs=3)
                                eng = nc.vector if diag else nc.gpsimd
                                eng.tensor_add(ps[:], pr[:, 0, :], pr[:, 1, :])
                                nc.vector.tensor_add(acc[:], acc[:], ps[:])
                            if prev is not None:
                                pp, ppr = prev
                                for h2 in range(2):
                                    tkl = 2 * pp + h2 - 4 * ts
                                    a0 = 128 * tkl if tkl > 0 else 0
                                    nc.tensor.matmul(
                                        op[:, a0:], v_t[b][:, 2 * pp + h2, hh, :],
                                        ppr[:, h2, a0:],
                                        start=(pp == 0 and h2 == 0), stop=False)
                            prev = (p, pr)
                        pp, ppr = prev
                        for h2 in range(2):
                            tkl = 2 * pp + h2 - 4 * ts
                            a0 = 128 * tkl if tkl > 0 else 0
                            nc.tensor.matmul(
                                op[:, a0:], v_t[b][:, 2 * pp + h2, hh, :],
                                ppr[:, h2, a0:],
                                start=(pp == 0 and h2 == 0), stop=(h2 == 1))
                        pend = (op, acc, hh, ts)
                    # flush before the collective: it needs every tile's aos
                    epilogue(pend)
                    pend = None
                    nc.gpsimd.collective_compute(
                        "AllToAll", mybir.AluOpType.bypass,
                        replica_groups=[list(range(NC_))],
                        ins=[a2a_in[hh].opt()], outs=[a2a_out[hh].opt()])
                return a2a_out

            def load_aog(b, a2a_out, aogp):
                # prefetch a2a results into SBUF in 128KB chunks right after
                # the collectives are issued; runs under the following phase.
                aoG = [aogp.tile([128, NC_, ROWS], BF16, tag=f"aoG{b}{hh}",
                                 name=f"aoG{b}{hh}") for hh in range(HPC)]
                for hh in range(HPC):
                    src = a2a_out[hh][:, :, :].rearrange("s d q -> d s q")
                    for c in range(4):
                        nc.sync.dma_start(out=aoG[hh][:, 2 * c:2 * c + 2, :],
                                          in_=src[:, 2 * c:2 * c + 2, :])
                return aoG

            def outproj(b, aoG, yop, yps):
                # aoG[hh][d, src, q] == head (2*src+hh) for my ROWS of batch b
                # tile-at-a-time: each (tb, nb) chunk accumulates its 16
                # head contributions back-to-back, then evacuates while the
                # next chunk computes — output writes pipeline instead of
                # bursting at the end.
                for tb in range(2):
                    for nb in range(4):
                        yp = yps.tile([128, TS], F32, tag="yp", bufs=4)
                        for hh in range(HPC):
                            for s in range(NC_):
                                nc.tensor.matmul(
                                    yp[:], aoG[hh][:, s, tb * 128:(tb + 1) * 128],
                                    wo_t[:, 2 * s + hh, nb * TS:(nb + 1) * TS],
                                    start=(hh == 0 and s == 0),
                                    stop=(hh == 1 and s == NC_ - 1))
                        yo = yop.tile([128, TS], F32, tag="yo", bufs=4)
                        nc.vector.tensor_add(yo[:], yp[:],
                                             bo_t[:, nb * TS:(nb + 1) * TS])
                        # all output writes ride the scalar queue: the sync
                        # queue is parked on the next batch's a2a_out waits
                        # and would stall yo buffer recycling.
                        for c in range(4):
                            nc.scalar.dma_start(
                                out=out_d.ap()[b, tb * 128:(tb + 1) * 128,
                                               nb * TS + c * 128:
                                               nb * TS + (c + 1) * 128],
                                in_=yo[:, c * 128:(c + 1) * 128])

            # ---- phase 1: QKV projections for both batches (no collectives)
            with tc.tile_pool(name="s1w", bufs=1) as s1w, \
                    tc.tile_pool(name="xp", bufs=1) as xp, \
                    tc.tile_pool(name="qep", bufs=1) as qep, \
                    tc.tile_pool(name="tmp", bufs=1) as tmp, \
                    tc.tile_pool(name="s1ps", bufs=1, space="PSUM") as s1ps:
                wqk_t = [s1w.tile([128, 4 * 128], BF16, tag=f"wqk{ci}",
                                  name=f"wqk{ci}") for ci in range(NCH)]
                wv_t = [s1w.tile([128, 2 * 128], BF16, tag=f"wv{ci}",
                                 name=f"wv{ci}") for ci in range(NCH)]

                def new_xt():
                    return xp.tile([128, TS], BF16, tag="xt", name="xt", bufs=16)

                # head: the (b0, ts0) stage-in fans out over three queues so
                # the first matmul waits on exactly one weight chunk + one x
                # tile (256KB), not the whole stage-in.
                xt0 = []
                for ci in range(NCH):
                    nc.scalar.dma_start(out=wqk_t[ci], in_=wqk_r[:, ci, :])
                    nc.gpsimd.dma_start(out=wv_t[ci], in_=wv_r[:, ci, :])
                    xt = new_xt()
                    nc.sync.dma_start(
                        out=xt, in_=xT_d.ap()[0, ci * 128:(ci + 1) * 128, 0:TS])
                    xt0.append(xt)
                load_consts()
                for b in range(B):
                    stage1(b, xt0, new_xt, qep, tmp, s1ps)

            # wo streams in while attention runs (DMA rings are idle there)
            for ci in range(NCH):
                nc.sync.dma_start(out=wo_t[:, ci, :], in_=wo_r[:, ci, :])

            # ---- phase 2: attention + exchanges --------------------------
            a2a_outs = {}
            aoGs = {}
            npairs_done = [0]
            with tc.tile_pool(name="aogp", bufs=1) as aogp:
                with tc.tile_pool(name="atps", bufs=1, space="PSUM") as atps, \
                        tc.tile_pool(name="prp", bufs=1) as prp, \
                        tc.tile_pool(name="accp", bufs=1) as accp, \
                        tc.tile_pool(name="bsp", bufs=1) as bsp, \
                        tc.tile_pool(name="aosp", bufs=1) as aosp:
                    # pre-allocate PSUM tags in bank order; 'warm' lands on
                    # the spare 8th bank, hosting throwaway warm-up matmuls.
                    for _ in range(2):
                        atps.tile([128, 2, TS], F32, tag="st", bufs=2,
                                  name="st")
                        atps.tile([128, TS], F32, tag="op", bufs=2, name="op")
                    atps.tile([128, TS], F32, tag="sm", bufs=1, name="sm")
                    warm = atps.tile([64, 64], F32, tag="warm", bufs=1,
                                     name="warm")
                    # bridge the QKV->attention hand-off: dependency-free
                    # dummies sit at this program position in the PE stream
                    # (the scheduler is priority-ordered), so they fill the
                    # eviction-chain bubble and keep HAM at full clock.
                    for _ in range(90):
                        nc.tensor.matmul(warm[:], ones_b[:, 0:64],
                                         ones_b[:, 0:64],
                                         start=True, stop=True)
                    for b in range(B):
                        a2a_outs[b] = attention(b, atps, prp, accp, bsp, aosp,
                                                warm, npairs_done)
                        aoGs[b] = load_aog(b, a2a_outs[b], aogp)
                    # bridge the attention->outproj hand-off the same way
                    for _ in range(40):
                        nc.tensor.matmul(warm[:], ones_b[:, 0:64],
                                         ones_b[:, 0:64],
                                         start=True, stop=True)

                # ---- phase 3: output projections -------------------------
                with tc.tile_pool(name="yps", bufs=1, space="PSUM") as yps, \
                        tc.tile_pool(name="yop", bufs=1) as yop:
                    for b in range(B):
                        outproj(b, aoGs[b], yop, yps)

    nc.compile()
    return nc


_NC_CACHE = None


def _get_program():
    global _NC_CACHE
    if _NC_CACHE is None:
        _NC_CACHE = _build_program()
    return _NC_CACHE


def make_in_maps(x, rope_cos, rope_sin, qkv_w, qkv_b, out_w, out_b):
    x = np.asarray(x, dtype=np.float32)
    qkv_w = np.asarray(qkv_w, dtype=np.float32)
    qkv_b = np.asarray(qkv_b, dtype=np.float32)
    out_w = np.asarray(out_w, dtype=np.float32)
    out_b = np.asarray(out_b, dtype=np.float32)

    xT = np.ascontiguousarray(x.transpose(0, 2, 1)).astype(BF16_NP)  # [B, D, T]
    cosT = np.ascontiguousarray(np.asarray(rope_cos, np.float32)[0, 0].T).astype(BF16_NP)
    sinTs = np.ascontiguousarray(np.asarray(rope_sin, np.float32)[0, 0].T).copy()
    sinTs[0:64, :] *= -1.0
    sinTs = sinTs.astype(BF16_NP)

    tk_idx = np.arange(128)[:, None]
    tq_idx = np.arange(TS)[None, :]
    masks = np.stack(
        [np.where(mi * 128 + tk_idx <= tq_idx, 1.0, 0.0) for mi in range(4)]
    ).astype(BF16_NP)                                           # [4, 128, TS]
    ones = np.ones((128, 128), BF16_NP)
    wo = np.ascontiguousarray(out_w.T).astype(BF16_NP)          # [D, D]
    bo = out_b.reshape(1, D)

    in_maps = []
    for c in range(NC_):
        h0 = HPC * c
        qr = qkv_w[h0 * 128:(h0 + HPC) * 128]                  # [256, D]
        kr = qkv_w[D + h0 * 128:D + (h0 + HPC) * 128]
        vr = qkv_w[2 * D + h0 * 128:2 * D + (h0 + HPC) * 128]
        wqk = np.ascontiguousarray(np.concatenate([qr, kr], 0).T).astype(BF16_NP)
        wv = np.ascontiguousarray(vr.T).astype(BF16_NP)        # [D, 256]
        bqk = np.stack(
            [qkv_b[h0 * 128:(h0 + 1) * 128],
             qkv_b[(h0 + 1) * 128:(h0 + 2) * 128],
             qkv_b[D + h0 * 128:D + (h0 + 1) * 128],
             qkv_b[D + (h0 + 1) * 128:D + (h0 + 2) * 128]], axis=1)  # [128, 4]
        bv = qkv_b[2 * D + h0 * 128:2 * D + (h0 + HPC) * 128].reshape(1, 256)
        in_maps.append({
            "xT": xT, "wqk": wqk, "wv": wv, "wo": wo,
            "cosT": cosT, "sinTs": sinTs, "masks": masks, "ones": ones,
            "bqk": np.ascontiguousarray(bqk),
            "bv": np.ascontiguousarray(bv).astype(BF16_NP),
            "bo": bo,
        })
    return in_maps


def assemble(results):
    y = np.empty((B, T, D), dtype=np.float32)
    for c in range(NC_):
        y[:, c * ROWS:(c + 1) * ROWS, :] = results[c]["out"]
    return y


def run(inputs, trace=False, trace_cores=None):
    nc = _get_program()
    in_maps = make_in_maps(**inputs)
    res = run_bass_kernel_spmd(
        nc, in_maps, list(range(NC_)), trace=trace,
        trace_cores=trace_cores if trace else None)
    return assemble(res.results), res


def kernel(**inputs) -> np.ndarray:
    y, _ = run(inputs, trace=False)
    return y


# revision 19
# speedup vs baseline: 1.0734x; 1.0153x over previous
"""Causal self-attention (B=2, T=2048, D=2048, H=16, d=128) on 8 TRN2 NeuronCores.

Sharding: head-parallel compute, token-parallel output. Core c owns heads
{2c, 2c+1} for both batches: column-parallel QKV projection, per-head RoPE +
causal attention. The per-head attention outputs are exchanged with one
AllToAll per (batch, head), after which every core holds all 16 heads for its
own 256 rows and computes the full output projection locally. Host
concatenates the 8 contiguous row shards.

v3 schedule notes (changes vs v2 baseline):
  - Head: per-ci weight/x DMAs fan out over three engine queues (sync/vector/
    scalar) with per-ci weight TILES so the first matmul waits on exactly two
    128KB transfers, not the whole 5MB stage-in.
  - wo (out_w) streams in during the attention phase instead of colliding
    with the ts1/ts2 x prefetches.
  - Attention: causal diagonal pairs use a pair-uniform column offset
    q0 = 128*(2p-4ts) (also at ts=0); the mask multiply runs on GpSimd and
    the softmax reciprocal on ACT to keep DVE off the critical path.
  - a2a_out -> SBUF (aoG) loads are issued right after each batch's
    collectives (prefetched under the next compute phase) and split into
    128KB chunks so they spread across DMA rings.
  - Output DMA is split into 64KB chunks alternating between two queues.
  - A short stream of throwaway N=64 matmuls bridges the QKV->attention
    transition so the PE clock-gate (HAM) never sees an idle window.
Matmuls run bf16 (1cyc/row); accumulation fp32 in PSUM.
"""
import math
import numpy as np
import ml_dtypes
from contextlib import ExitStack

import concourse.bass as bass
import concourse.tile as tile
from concourse import bacc, mybir
from concourse.bass_utils import run_bass_kernel_spmd

F32 = mybir.dt.float32
BF16 = mybir.dt.bfloat16
BF16_NP = ml_dtypes.bfloat16
AF = mybir.ActivationFunctionType
ALU = mybir.AluOpType

NC_ = 8           # cores
B, T, D = 2, 2048, 2048
H, HD = 16, 128   # heads, head_dim
HPC = H // NC_    # heads per core = 2
TS = 512          # t-super tile
NTS = T // TS     # 4
NCH = D // 128    # 16 contraction chunks
ROWS = T // NC_   # 256 own token rows per batch
SCALE = 1.0 / math.sqrt(HD)


def _build_program():
    nc = bacc.Bacc("TRN2", target_bir_lowering=False, debug=False, num_devices=NC_)

    xT_d = nc.dram_tensor("xT", [B, D, T], BF16, kind="ExternalInput")
    wqk_d = nc.dram_tensor("wqk", [D, 4 * 128], BF16, kind="ExternalInput")
    wv_d = nc.dram_tensor("wv", [D, 2 * 128], BF16, kind="ExternalInput")
    wo_d = nc.dram_tensor("wo", [D, D], BF16, kind="ExternalInput")
    cos_d = nc.dram_tensor("cosT", [128, T], BF16, kind="ExternalInput")
    sin_d = nc.dram_tensor("sinTs", [128, T], BF16, kind="ExternalInput")
    mask_d = nc.dram_tensor("masks", [4, 128, TS], BF16, kind="ExternalInput")
    ones_d = nc.dram_tensor("ones", [128, 128], BF16, kind="ExternalInput")
    bqk_d = nc.dram_tensor("bqk", [128, 4], F32, kind="ExternalInput")
    bv_d = nc.dram_tensor("bv", [1, 2 * 128], BF16, kind="ExternalInput")
    bo_d = nc.dram_tensor("bo", [1, D], F32, kind="ExternalInput")
    out_d = nc.dram_tensor("out", [B, ROWS, D], F32, kind="ExternalOutput")

    with tile.TileContext(nc) as tc:
        with ExitStack() as ctx:
            consts = ctx.enter_context(tc.tile_pool(name="consts", bufs=1))
            qkv = ctx.enter_context(tc.tile_pool(name="qkv", bufs=1))
            dramp = ctx.enter_context(tc.tile_pool(name="dramp", bufs=1, space="DRAM"))

            wqk_r = wqk_d.ap().rearrange("(c p) e -> p c e", p=128)
            wv_r = wv_d.ap().rearrange("(c p) e -> p c e", p=128)
            wo_r = wo_d.ap().rearrange("(h p) o -> p h o", p=128)

            cos_t = consts.tile([128, T], BF16)
            sin_t = consts.tile([128, T], BF16)
            bqk_t = consts.tile([128, 4], F32)
            ones_b = consts.tile([128, 128], BF16)
            mask_t = consts.tile([128, 4, TS], BF16)
            bv_t = consts.tile([128, 2 * 128], BF16)
            bo_t = consts.tile([128, D], F32)
            wo_t = consts.tile([128, H, D], BF16)

            def load_consts():
                # consts follow the critical (b0, ts0) stage-in on the
                # scalar/vector queues
                for q in range(4):
                    nc.scalar.dma_start(out=cos_t[:, q * TS:(q + 1) * TS],
                                        in_=cos_d.ap()[:, q * TS:(q + 1) * TS])
                    nc.scalar.dma_start(out=sin_t[:, q * TS:(q + 1) * TS],
                                        in_=sin_d.ap()[:, q * TS:(q + 1) * TS])
                nc.scalar.dma_start(out=bqk_t, in_=bqk_d.ap())
                nc.scalar.dma_start(out=ones_b, in_=ones_d.ap())
                nc.gpsimd.dma_start(out=mask_t,
                                    in_=mask_d.ap().rearrange("m p n -> p m n"))
                nc.gpsimd.dma_start(out=bv_t,
                                    in_=bv_d.ap().partition_broadcast(128))
                nc.gpsimd.dma_start(out=bo_t,
                                    in_=bo_d.ap().partition_broadcast(128))

            q_t = {b: qkv.tile([128, HPC, T], BF16, tag=f"q{b}", name=f"q_t{b}")
                   for b in range(B)}
            k_t = {b: qkv.tile([128, HPC, T], BF16, tag=f"k{b}", name=f"k_t{b}")
                   for b in range(B)}
            v_t = {b: qkv.tile([128, NTS * 4, HPC, 128], BF16, tag=f"v{b}",
                               name=f"v_t{b}") for b in range(B)}

            def stage1(b, xt0, new_xt, qep, tmp, s1ps):
                for ts in range(NTS):
                    qkp = [s1ps.tile([128, TS], F32, tag=f"qkp{j}", name=f"qkp{j}")
                           for j in range(4)]
                    vp = [s1ps.tile([128, 2 * 128], F32, tag=f"vp{tb}",
                                    name=f"vp{tb}")[:] for tb in range(4)]
                    last_tile = (b == B - 1 and ts == NTS - 1)
                    xts = []
                    for ci in range(NCH):
                        if b == 0 and ts == 0:
                            xt = xt0[ci]
                        else:
                            xt = new_xt()
                            nc.sync.dma_start(
                                out=xt,
                                in_=xT_d.ap()[b, ci * 128:(ci + 1) * 128,
                                              ts * TS:(ts + 1) * TS],
                            )
                        xts.append(xt)
                        st_, sp_ = ci == 0, ci == NCH - 1
                        for j in range(4):
                            nc.tensor.matmul(
                                qkp[j][:], wqk_t[ci][:, j * 128:(j + 1) * 128],
                                xt[:], start=st_, stop=sp_)
                        if not last_tile:
                            for tb in range(4):
                                nc.tensor.matmul(
                                    vp[tb], xt[:, tb * 128:(tb + 1) * 128],
                                    wv_t[ci][:], start=st_, stop=sp_)
                    if last_tile:
                        # final tile runs qk first, v second: the 4-5us RoPE
                        # eviction chain then overlaps the v matmuls, so the
                        # s1ps pool releases ~2us after the last matmul
                        # instead of ~6 (the attention PSUM pool overlays it).
                        for ci in range(NCH):
                            st_, sp_ = ci == 0, ci == NCH - 1
                            for tb in range(4):
                                nc.tensor.matmul(
                                    vp[tb], xts[ci][:, tb * 128:(tb + 1) * 128],
                                    wv_t[ci][:], start=st_, stop=sp_)
                    # evict q/k to bf16 on ACT (plus a half-swapped copy for
                    # rotate_half); RoPE + bias fused on DVE. sinTs rows 0:64
                    # carry the rotate_half sign flip.
                    cs = cos_t[:, ts * TS:(ts + 1) * TS]
                    sn = sin_t[:, ts * TS:(ts + 1) * TS]
                    for j in range(4):
                        qe = qep.tile([128, TS], BF16, tag=f"qe{j}", name=f"qe{j}",
                                      bufs=3)
                        qs = qep.tile([128, TS], BF16, tag=f"qs{j}", name=f"qs{j}",
                                      bufs=2)
                        nc.scalar.activation(qe[:], qkp[j][:], AF.Copy)
                        nc.scalar.activation(qs[0:64, :], qe[64:128, :], AF.Copy)
                        nc.scalar.activation(qs[64:128, :], qe[0:64, :], AF.Copy)
                        t1 = tmp.tile([128, TS], BF16, tag="t1", bufs=2)
                        t2 = tmp.tile([128, TS], BF16, tag="t2", bufs=2)
                        nc.vector.tensor_mul(t1[:], qe[:], cs)
                        nc.vector.tensor_mul(t2[:], qs[:], sn)
                        dst = (q_t[b] if j < 2 else k_t[b])[:, j % 2,
                                                            ts * TS:(ts + 1) * TS]
                        nc.vector.scalar_tensor_tensor(
                            dst, t1[:], bqk_t[:, j:j + 1], t2[:], ALU.add, ALU.add)
                    for tb in range(4):
                        vdst = v_t[b][:, ts * 4 + tb, :, :]
                        nc.scalar.activation(
                            vdst, vp[tb].rearrange("p (h e) -> p h e", h=HPC),
                            AF.Copy)
                        nc.vector.tensor_add(
                            vdst, vdst,
                            bv_t[:].rearrange("p (h e) -> p h e", h=HPC))

            def attention(b, atps, prp, accp, bsp, aosp, warm, npairs_done):
                # one AllToAll per (b, head); triggered as soon as that head's
                # normalized outputs are in DRAM. The per-tile epilogue
                # (denominator matmul, reciprocal, normalize, DRAM write) is
                # deferred until the next tile's first score pair so the PE
                # never waits on the ACT/gpsimd accumulation chain.
                a2a_in = [dramp.tile([NC_, 128, ROWS], BF16, tag=f"a2i{b}{hh}",
                                     name=f"a2i{b}{hh}") for hh in range(HPC)]
                a2a_out = [dramp.tile([NC_, 128, ROWS], BF16, tag=f"a2o{b}{hh}",
                                      name=f"a2o{b}{hh}") for hh in range(HPC)]

                def epilogue(pend):
                    op, acc, hh, ts = pend
                    sm = atps.tile([128, TS], F32, tag="sm", bufs=1)
                    nc.tensor.matmul(sm[:], ones_b[:], acc[:], start=True,
                                     stop=True)
                    bsb = bsp.tile([128, TS], F32, tag="bsb", bufs=2)
                    with nc.allow_low_precision(reason="softmax recip"):
                        nc.vector.reciprocal_approx_fast(bsb[:], sm[:])
                    aos = aosp.tile([128, TS], BF16, tag="aos", bufs=4)
                    nc.vector.tensor_mul(aos[:], op[:], bsb[:])
                    nc.gpsimd.dma_start(
                        out=a2a_in[hh][2 * ts:2 * ts + 2, :, :].transpose(
                            [1, 0, 2]),
                        in_=aos[:].rearrange("d (s q) -> d s q", s=2))

                pend = None
                for hh in range(HPC):
                    for ts in range(NTS):
                        op = None
                        acc = accp.tile([128, TS], BF16, tag="acc", bufs=2)
                        npair = 2 * (ts + 1)
                        prev = None
                        for p in range(npair):
                            st = atps.tile([128, 2, TS], F32, tag="st", bufs=2)
                            # diagonal pairs: skip the columns whose queries
                            # sit fully below every key tile of the pair. The
                            # mask multiply zeroes the skipped region.
                            diag = p >= 2 * ts
                            q0 = max(0, 128 * (2 * p - 4 * ts)) if diag else 0
                            for h2 in range(2):
                                tk = 2 * p + h2
                                nc.tensor.matmul(
                                    st[:, h2, q0:],
                                    k_t[b][:, hh, tk * 128:(tk + 1) * 128],
                                    q_t[b][:, hh,
                                          ts * TS + q0:(ts + 1) * TS],
                                    start=True, stop=True)
                            if op is None:
                                op = atps.tile([128, TS], F32, tag="op", bufs=2)
                            if p == 1 and pend is not None:
                                epilogue(pend)
                                pend = None
                            pr = prp.tile([128, 2, TS], BF16, tag="pr", bufs=4)
                            # first 4 pairs: pr buffers are uninitialized
                            # SBUF; exp full-width so no stale bits (possibly
                            # NaN) survive into the masked multiply.
                            qe0 = q0 if npairs_done[0] >= 4 else 0
                            nc.scalar.activation(pr[:, :, qe0:], st[:, :, qe0:],
                                                 AF.Exp, scale=SCALE)
                            npairs_done[0] += 1
                            if diag:  # zero masked scores (and skipped cols)
                                mi = p - 2 * ts
                                nc.vector.tensor_mul(
                                    pr[:], pr[:], mask_t[:, 2 * mi:2 * mi + 2, :])
                            if p == 0:
                                nc.vector.tensor_add(acc[:], pr[:, 0, :],
                                                     pr[:, 1, :])
                            else:
                                ps = bsp.tile([128, TS], BF16, tag="ps", bufs=3)
                                nc.vector.tensor_add(ps[:], pr[:, 0, :],
                                                     pr[:, 1, :])
                                nc.vector.tensor_add(acc[:], acc[:], ps[:])
                            if prev is not None:
                                pp, ppr = prev
                                for h2 in range(2):
                                    tkl = 2 * pp + h2 - 4 * ts
                                    a0 = 128 * tkl if tkl > 0 else 0
                                    nc.tensor.matmul(
                                        op[:, a0:], v_t[b][:, 2 * pp + h2, hh, :],
                                        ppr[:, h2, a0:],
                                        start=(pp == 0 and h2 == 0), stop=False)
                            prev = (p, pr)
                        pp, ppr = prev
                        for h2 in range(2):
                            tkl = 2 * pp + h2 - 4 * ts
                            a0 = 128 * tkl if tkl > 0 else 0
                            nc.tensor.matmul(
                                op[:, a0:], v_t[b][:, 2 * pp + h2, hh, :],
                                ppr[:, h2, a0:],
                                start=(pp == 0 and h2 == 0), stop=(h2 == 1))
                        pend = (op, acc, hh, ts)
                    # flush before the collective: it needs every tile's aos
                    epilogue(pend)
                    pend = None
                    nc.gpsimd.collective_compute(
                        "AllToAll", mybir.AluOpType.bypass,
                        replica_groups=[list(range(NC_))],
                        ins=[a2a_in[hh].opt()], outs=[a2a_out[hh].opt()])
                return a2a_out

            def load_aog(b, a2a_out, aogp):
                # prefetch a2a results into SBUF in 128KB chunks right after
                # the collectives are issued; runs under the following phase.
                aoG = [aogp.tile([128, NC_, ROWS], BF16, tag=f"aoG{b}{hh}",
                                 name=f"aoG{b}{hh}") for hh in range(HPC)]
                for hh in range(HPC):
                    src = a2a_out[hh][:, :, :].rearrange("s d q -> d s q")
                    for c in range(4):
                        nc.sync.dma_start(out=aoG[hh][:, 2 * c:2 * c + 2, :],
                                          in_=src[:, 2 * c:2 * c + 2, :])
                return aoG

            def outproj(b, aoG, yop, yps):
                # aoG[hh][d, src, q] == head (2*src+hh) for my ROWS of batch b
                # tile-at-a-time: each (tb, nb) chunk accumulates its 16
                # head contributions back-to-back, then evacuates while the
                # next chunk computes — output writes pipeline instead of
                # bursting at the end.
                for tb in range(2):
                    for nb in range(4):
                        yp = yps.tile([128, TS], F32, tag="yp", bufs=4)
                        for hh in range(HPC):
                            for s in range(NC_):
                                nc.tensor.matmul(
                                    yp[:], aoG[hh][:, s, tb * 128:(tb + 1) * 128],
                                    wo_t[:, 2 * s + hh, nb * TS:(nb + 1) * TS],
                                    start=(hh == 0 and s == 0),
                                    stop=(hh == 1 and s == NC_ - 1))
                        yo = yop.tile([128, TS], F32, tag="yo", bufs=4)
                        nc.vector.tensor_add(yo[:], yp[:],
                                             bo_t[:, nb * TS:(nb + 1) * TS])
                        # all output writes ride the scalar queue: the sync
                        # queue is parked on the next batch's a2a_out waits
                        # and would stall yo buffer recycling.
                        for c in range(4):
                            nc.scalar.dma_start(
                                out=out_d.ap()[b, tb * 128:(tb + 1) * 128,
                                               nb * TS + c * 128:
                                               nb * TS + (c + 1) * 128],
                                in_=yo[:, c * 128:(c + 1) * 128])

            # ---- phase 1: QKV projections for both batches (no collectives)
            with tc.tile_pool(name="s1w", bufs=1) as s1w, \
                    tc.tile_pool(name="xp", bufs=1) as xp, \
                    tc.tile_pool(name="qep", bufs=1) as qep, \
                    tc.tile_pool(name="tmp", bufs=1) as tmp, \
                    tc.tile_pool(name="s1ps", bufs=1, space="PSUM") as s1ps:
                wqk_t = [s1w.tile([128, 4 * 128], BF16, tag=f"wqk{ci}",
                                  name=f"wqk{ci}") for ci in range(NCH)]
                wv_t = [s1w.tile([128, 2 * 128], BF16, tag=f"wv{ci}",
                                 name=f"wv{ci}") for ci in range(NCH)]

                def new_xt():
                    return xp.tile([128, TS], BF16, tag="xt", name="xt", bufs=16)

                # head: the (b0, ts0) stage-in fans out over three queues so
                # the first matmul waits on exactly one weight chunk + one x
                # tile (256KB), not the whole stage-in.
                xt0 = []
                for ci in range(NCH):
                    nc.scalar.dma_start(out=wqk_t[ci], in_=wqk_r[:, ci, :])
                    nc.gpsimd.dma_start(out=wv_t[ci], in_=wv_r[:, ci, :])
                    xt = new_xt()
                    nc.sync.dma_start(
                        out=xt, in_=xT_d.ap()[0, ci * 128:(ci + 1) * 128, 0:TS])
                    xt0.append(xt)
                load_consts()
                for b in range(B):
                    stage1(b, xt0, new_xt, qep, tmp, s1ps)

            # wo streams in while attention runs (DMA rings are idle there)
            for ci in range(NCH):
                nc.sync.dma_start(out=wo_t[:, ci, :], in_=wo_r[:, ci, :])

            # ---- phase 2: attention + exchanges --------------------------
            a2a_outs = {}
            aoGs = {}
            npairs_done = [0]
            with tc.tile_pool(name="aogp", bufs=1) as aogp:
                with tc.tile_pool(name="atps", bufs=1, space="PSUM") as atps, \
                        tc.tile_pool(name="prp", bufs=1) as prp, \
                        tc.tile_pool(name="accp", bufs=1) as accp, \
                        tc.tile_pool(name="bsp", bufs=1) as bsp, \
                        tc.tile_pool(name="aosp", bufs=1) as aosp:
                    # pre-allocate PSUM tags in bank order; 'warm' lands on
                    # the spare 8th bank, hosting throwaway warm-up matmuls.
                    for _ in range(2):
                        atps.tile([128, 2, TS], F32, tag="st", bufs=2,
                                  name="st")
                        atps.tile([128, TS], F32, tag="op", bufs=2, name="op")
                    atps.tile([128, TS], F32, tag="sm", bufs=1, name="sm")
                    warm = atps.tile([64, 64], F32, tag="warm", bufs=1,
                                     name="warm")
                    # bridge the QKV->attention hand-off: dependency-free
                    # dummies sit at this program position in the PE stream
                    # (the scheduler is priority-ordered), so they fill the
                    # eviction-chain bubble and keep HAM at full clock.
                    for _ in range(90):
                        nc.tensor.matmul(warm[:], ones_b[:, 0:64],
                                         ones_b[:, 0:64],
                                         start=True, stop=True)
                    for b in range(B):
                        a2a_outs[b] = attention(b, atps, prp, accp, bsp, aosp,
                                                warm, npairs_done)
                        aoGs[b] = load_aog(b, a2a_outs[b], aogp)
                    # bridge the attention->outproj hand-off the same way
                    for _ in range(40):
                        nc.tensor.matmul(warm[:], ones_b[:, 0:64],
                                         ones_b[:, 0:64],
                                         start=True, stop=True)

                # ---- phase 3: output projections -------------------------
                with tc.tile_pool(name="yps", bufs=1, space="PSUM") as yps, \
                        tc.tile_pool(name="yop", bufs=1) as yop:
                    for b in range(B):
                        outproj(b, aoGs[b], yop, yps)

    nc.compile()
    return nc


_NC_CACHE = None


def _get_program():
    global _NC_CACHE
    if _NC_CACHE is None:
        _NC_CACHE = _build_program()
    return _NC_CACHE


def make_in_maps(x, rope_cos, rope_sin, qkv_w, qkv_b, out_w, out_b):
    x = np.asarray(x, dtype=np.float32)
    qkv_w = np.asarray(qkv_w, dtype=np.float32)
    qkv_b = np.asarray(qkv_b, dtype=np.float32)
    out_w = np.asarray(out_w, dtype=np.float32)
    out_b = np.asarray(out_b, dtype=np.float32)

    xT = np.ascontiguousarray(x.transpose(0, 2, 1)).astype(BF16_NP)  # [B, D, T]
    cosT = np.ascontiguousarray(np.asarray(rope_cos, np.float32)[0, 0].T).astype(BF16_NP)
    sinTs = np.ascontiguousarray(np.asarray(rope_sin, np.float32)[0, 0].T).copy()
    sinTs[0:64, :] *= -1.0
    sinTs = sinTs.astype(BF16_NP)

    tk_idx = np.arange(128)[:, None]
    tq_idx = np.arange(TS)[None, :]
    masks = np.stack(
        [np.where(mi * 128 + tk_idx <= tq_idx, 1.0, 0.0) for mi in range(4)]
    ).astype(BF16_NP)                                           # [4, 128, TS]
    ones = np.ones((128, 128), BF16_NP)
    wo = np.ascontiguousarray(out_w.T).astype(BF16_NP)          # [D, D]
    bo = out_b.reshape(1, D)

    in_maps = []
    for c in range(NC_):
        h0 = HPC * c
        qr = qkv_w[h0 * 128:(h0 + HPC) * 128]                  # [256, D]
        kr = qkv_w[D + h0 * 128:D + (h0 + HPC) * 128]
        vr = qkv_w[2 * D + h0 * 128:2 * D + (h0 + HPC) * 128]
        wqk = np.ascontiguousarray(np.concatenate([qr, kr], 0).T).astype(BF16_NP)
        wv = np.ascontiguousarray(vr.T).astype(BF16_NP)        # [D, 256]
        bqk = np.stack(
            [qkv_b[h0 * 128:(h0 + 1) * 128],
             qkv_b[(h0 + 1) * 128:(h0 + 2) * 128],
             qkv_b[D + h0 * 128:D + (h0 + 1) * 128],
             qkv_b[D + (h0 + 1) * 128:D + (h0 + 2) * 128]], axis=1)  # [128, 4]
        bv = qkv_b[2 * D + h0 * 128:2 * D + (h0 + HPC) * 128].reshape(1, 256)
        in_maps.append({
            "xT": xT, "wqk": wqk, "wv": wv, "wo": wo,
            "cosT": cosT, "sinTs": sinTs, "masks": masks, "ones": ones,
            "bqk": np.ascontiguousarray(bqk),
            "bv": np.ascontiguousarray(bv).astype(BF16_NP),
            "bo": bo,
        })
    return in_maps


def assemble(results):
    y = np.empty((B, T, D), dtype=np.float32)
    for c in range(NC_):
        y[:, c * ROWS:(c + 1) * ROWS, :] = results[c]["out"]
    return y


def run(inputs, trace=False, trace_cores=None):
    nc = _get_program()
    in_maps = make_in_maps(**inputs)
    res = run_bass_kernel_spmd(
        nc, in_maps, list(range(NC_)), trace=trace,
        trace_cores=trace_cores if trace else None)
    return assemble(res.results), res


def kernel(**inputs) -> np.ndarray:
    y, _ = run(inputs, trace=False)
    return y


# revision 26
# speedup vs baseline: 1.1356x; 1.0579x over previous
"""Causal self-attention (B=2, T=2048, D=2048, H=16, d=128) on 8 TRN2 NeuronCores.

Sharding: head-parallel compute, token-parallel output. Core c owns heads
{2c, 2c+1} for both batches: column-parallel QKV projection, per-head RoPE +
causal attention. The per-head attention outputs are exchanged with one
AllToAll per (batch, head), after which every core holds all 16 heads for its
own 256 rows and computes the full output projection locally. Host
concatenates the 8 contiguous row shards.

v3 schedule notes (changes vs v2 baseline):
  - Head: per-ci weight/x DMAs fan out over three engine queues (sync/vector/
    scalar) with per-ci weight TILES so the first matmul waits on exactly two
    128KB transfers, not the whole 5MB stage-in.
  - wo (out_w) streams in during the attention phase instead of colliding
    with the ts1/ts2 x prefetches.
  - Attention: causal diagonal pairs use a pair-uniform column offset
    q0 = 128*(2p-4ts) (also at ts=0); the mask multiply runs on GpSimd and
    the softmax reciprocal on ACT to keep DVE off the critical path.
  - a2a_out -> SBUF (aoG) loads are issued right after each batch's
    collectives (prefetched under the next compute phase) and split into
    128KB chunks so they spread across DMA rings.
  - Output DMA is split into 64KB chunks alternating between two queues.
  - A short stream of throwaway N=64 matmuls bridges the QKV->attention
    transition so the PE clock-gate (HAM) never sees an idle window.
Matmuls run bf16 (1cyc/row); accumulation fp32 in PSUM.
"""
import math
import numpy as np
import ml_dtypes
from contextlib import ExitStack

import concourse.bass as bass
import concourse.tile as tile
from concourse import bacc, mybir
from concourse.bass_utils import run_bass_kernel_spmd

F32 = mybir.dt.float32
BF16 = mybir.dt.bfloat16
BF16_NP = ml_dtypes.bfloat16
AF = mybir.ActivationFunctionType
ALU = mybir.AluOpType

NC_ = 8           # cores
B, T, D = 2, 2048, 2048
H, HD = 16, 128   # heads, head_dim
HPC = H // NC_    # heads per core = 2
TS = 512          # t-super tile
NTS = T // TS     # 4
NCH = D // 128    # 16 contraction chunks
ROWS = T // NC_   # 256 own token rows per batch
SCALE = 1.0 / math.sqrt(HD)


def _build_program():
    nc = bacc.Bacc("TRN2", target_bir_lowering=False, debug=False, num_devices=NC_)

    xT_d = nc.dram_tensor("xT", [B, D, T], BF16, kind="ExternalInput")
    wqk_d = nc.dram_tensor("wqk", [D, 4 * 128], BF16, kind="ExternalInput")
    wv_d = nc.dram_tensor("wv", [D, 2 * 128], BF16, kind="ExternalInput")
    wo_d = nc.dram_tensor("wo", [D, D], BF16, kind="ExternalInput")
    cos_d = nc.dram_tensor("cosT", [128, T], BF16, kind="ExternalInput")
    sin_d = nc.dram_tensor("sinTs", [128, T], BF16, kind="ExternalInput")
    mask_d = nc.dram_tensor("masks", [4, 128, TS], BF16, kind="ExternalInput")
    ones_d = nc.dram_tensor("ones", [128, 128], BF16, kind="ExternalInput")
    bqk_d = nc.dram_tensor("bqk", [128, 4], F32, kind="ExternalInput")
    bv_d = nc.dram_tensor("bv", [1, 2 * 128], BF16, kind="ExternalInput")
    bo_d = nc.dram_tensor("bo", [1, D], F32, kind="ExternalInput")
    out_d = nc.dram_tensor("out", [B, ROWS, D], F32, kind="ExternalOutput")

    with tile.TileContext(nc) as tc:
        with ExitStack() as ctx:
            consts = ctx.enter_context(tc.tile_pool(name="consts", bufs=1))
            qkv = ctx.enter_context(tc.tile_pool(name="qkv", bufs=1))
            dramp = ctx.enter_context(tc.tile_pool(name="dramp", bufs=1, space="DRAM"))

            wqk_r = wqk_d.ap().rearrange("(c p) e -> p c e", p=128)
            wv_r = wv_d.ap().rearrange("(c p) e -> p c e", p=128)
            wo_r = wo_d.ap().rearrange("(h p) o -> p h o", p=128)

            cos_t = consts.tile([128, T], BF16)
            sin_t = consts.tile([128, T], BF16)
            bqk_t = consts.tile([128, 4], F32)
            ones_b = consts.tile([128, 128], BF16)
            mask_t = consts.tile([128, 4, TS], BF16)
            bv_t = consts.tile([128, 2 * 128], BF16)
            bo_t = consts.tile([128, D], F32)
            wo_t = consts.tile([128, H, D], BF16)

            def load_consts():
                # consts follow the critical (b0, ts0) stage-in on the
                # scalar/vector queues
                for q in range(4):
                    nc.scalar.dma_start(out=cos_t[:, q * TS:(q + 1) * TS],
                                        in_=cos_d.ap()[:, q * TS:(q + 1) * TS])
                    nc.scalar.dma_start(out=sin_t[:, q * TS:(q + 1) * TS],
                                        in_=sin_d.ap()[:, q * TS:(q + 1) * TS])
                nc.scalar.dma_start(out=bqk_t, in_=bqk_d.ap())
                nc.scalar.dma_start(out=ones_b, in_=ones_d.ap())
                nc.gpsimd.dma_start(out=mask_t,
                                    in_=mask_d.ap().rearrange("m p n -> p m n"))
                nc.gpsimd.dma_start(out=bv_t,
                                    in_=bv_d.ap().partition_broadcast(128))
                nc.gpsimd.dma_start(out=bo_t,
                                    in_=bo_d.ap().partition_broadcast(128))

            q_t = {b: qkv.tile([128, HPC, T], BF16, tag=f"q{b}", name=f"q_t{b}")
                   for b in range(B)}
            k_t = {b: qkv.tile([128, HPC, T], BF16, tag=f"k{b}", name=f"k_t{b}")
                   for b in range(B)}
            v_t = {b: qkv.tile([128, NTS * 4, HPC, 128], BF16, tag=f"v{b}",
                               name=f"v_t{b}") for b in range(B)}

            def stage1(b, xt0, new_xt, qep, tmp, s1ps):
                for ts in range(NTS):
                    qkp = [s1ps.tile([128, TS], F32, tag=f"qkp{j}", name=f"qkp{j}")
                           for j in range(4)]
                    vp = [s1ps.tile([128, 2 * 128], F32, tag=f"vp{tb}",
                                    name=f"vp{tb}")[:] for tb in range(4)]
                    last_tile = (b == B - 1 and ts == NTS - 1)
                    xts = []
                    for ci in range(NCH):
                        if b == 0 and ts == 0:
                            xt = xt0[ci]
                        else:
                            xt = new_xt()
                            nc.sync.dma_start(
                                out=xt,
                                in_=xT_d.ap()[b, ci * 128:(ci + 1) * 128,
                                              ts * TS:(ts + 1) * TS],
                            )
                        xts.append(xt)
                        st_, sp_ = ci == 0, ci == NCH - 1
                        for j in range(4):
                            nc.tensor.matmul(
                                qkp[j][:], wqk_t[ci][:, j * 128:(j + 1) * 128],
                                xt[:], start=st_, stop=sp_)
                        if not last_tile:
                            for tb in range(4):
                                nc.tensor.matmul(
                                    vp[tb], xt[:, tb * 128:(tb + 1) * 128],
                                    wv_t[ci][:], start=st_, stop=sp_)
                    if last_tile:
                        # final tile runs qk first, v second: the 4-5us RoPE
                        # eviction chain then overlaps the v matmuls, so the
                        # s1ps pool releases ~2us after the last matmul
                        # instead of ~6 (the attention PSUM pool overlays it).
                        for ci in range(NCH):
                            st_, sp_ = ci == 0, ci == NCH - 1
                            for tb in range(4):
                                nc.tensor.matmul(
                                    vp[tb], xts[ci][:, tb * 128:(tb + 1) * 128],
                                    wv_t[ci][:], start=st_, stop=sp_)
                    # evict q/k to bf16 on ACT (plus a half-swapped copy for
                    # rotate_half); RoPE + bias fused on DVE. sinTs rows 0:64
                    # carry the rotate_half sign flip.
                    cs = cos_t[:, ts * TS:(ts + 1) * TS]
                    sn = sin_t[:, ts * TS:(ts + 1) * TS]
                    for j in range(4):
                        qe = qep.tile([128, TS], BF16, tag=f"qe{j}", name=f"qe{j}",
                                      bufs=3)
                        qs = qep.tile([128, TS], BF16, tag=f"qs{j}", name=f"qs{j}",
                                      bufs=2)
                        nc.scalar.activation(qe[:], qkp[j][:], AF.Copy)
                        nc.scalar.activation(qs[0:64, :], qe[64:128, :], AF.Copy)
                        nc.scalar.activation(qs[64:128, :], qe[0:64, :], AF.Copy)
                        t1 = tmp.tile([128, TS], BF16, tag="t1", bufs=2)
                        t2 = tmp.tile([128, TS], BF16, tag="t2", bufs=2)
                        nc.vector.tensor_mul(t1[:], qe[:], cs)
                        nc.vector.tensor_mul(t2[:], qs[:], sn)
                        dst = (q_t[b] if j < 2 else k_t[b])[:, j % 2,
                                                            ts * TS:(ts + 1) * TS]
                        nc.vector.scalar_tensor_tensor(
                            dst, t1[:], bqk_t[:, j:j + 1], t2[:], ALU.add, ALU.add)
                    for tb in range(4):
                        vdst = v_t[b][:, ts * 4 + tb, :, :]
                        nc.scalar.activation(
                            vdst, vp[tb].rearrange("p (h e) -> p h e", h=HPC),
                            AF.Copy)
                        nc.vector.tensor_add(
                            vdst, vdst,
                            bv_t[:].rearrange("p (h e) -> p h e", h=HPC))

            def attention(b, atps, prp, accp, bsp, aosp):
                # one AllToAll per (b, head); triggered as soon as that head's
                # normalized outputs are in DRAM. The per-tile epilogue
                # (denominator matmul, reciprocal, normalize, DRAM write) is
                # deferred until the next tile's first score pair so the PE
                # never waits on the ACT/gpsimd accumulation chain.
                a2a_in = dramp.tile([NC_, HPC, 128, ROWS], BF16, tag=f"a2i{b}",
                                    name=f"a2i{b}")
                a2a_out = dramp.tile([NC_, HPC, 128, ROWS], BF16, tag=f"a2o{b}",
                                     name=f"a2o{b}")

                def epilogue(pend):
                    op, acc, hh, ts = pend
                    sm = atps.tile([128, TS], F32, tag="sm", bufs=1)
                    nc.tensor.matmul(sm[:], ones_b[:], acc[:], start=True,
                                     stop=True)
                    bsb = bsp.tile([128, TS], F32, tag="bsb", bufs=2)
                    with nc.allow_low_precision(reason="softmax recip"):
                        nc.vector.reciprocal_approx_fast(bsb[:], sm[:])
                    aos = aosp.tile([128, TS], BF16, tag="aos", bufs=4)
                    nc.vector.tensor_mul(aos[:], op[:], bsb[:])
                    nc.gpsimd.dma_start(
                        out=a2a_in[2 * ts:2 * ts + 2, hh, :, :].transpose(
                            [1, 0, 2]),
                        in_=aos[:].rearrange("d (s q) -> d s q", s=2))

                pend = None
                for hh in range(HPC):
                    for ts in range(NTS):
                        op = None
                        acc = accp.tile([128, TS], BF16, tag="acc", bufs=2)
                        npair = 2 * (ts + 1)
                        prev = None
                        for p in range(npair):
                            st = atps.tile([128, 2, TS], F32, tag="st", bufs=2)
                            # diagonal pairs: skip the columns whose queries
                            # sit fully below every key tile of the pair. The
                            # mask multiply zeroes the skipped region.
                            diag = p >= 2 * ts
                            q0 = max(0, 128 * (2 * p - 4 * ts)) if diag else 0
                            for h2 in range(2):
                                tk = 2 * p + h2
                                nc.tensor.matmul(
                                    st[:, h2, q0:],
                                    k_t[b][:, hh, tk * 128:(tk + 1) * 128],
                                    q_t[b][:, hh,
                                          ts * TS + q0:(ts + 1) * TS],
                                    start=True, stop=True)
                            if op is None:
                                op = atps.tile([128, TS], F32, tag="op", bufs=2)
                            if p == 1 and pend is not None:
                                epilogue(pend)
                                pend = None
                            pr = prp.tile([128, 2, TS], BF16, tag="pr", bufs=4)
                            nc.scalar.activation(pr[:, :, q0:], st[:, :, q0:],
                                                 AF.Exp, scale=SCALE)
                            if diag:
                                # zero the masked scores. h2=0 only needs its
                                # block diagonal [q0, q0+128); h2=1 needs
                                # [q0, q0+256) (its fully-masked sub-block
                                # plus its diagonal). Columns below q0 stay
                                # stale but nothing downstream reads them:
                                # the pair-sum, acc and attnV are all
                                # windowed at >= q0.
                                mi = p - 2 * ts
                                m1 = min(q0 + 128, TS)
                                m2 = min(q0 + 256, TS)
                                nc.vector.tensor_mul(
                                    pr[:, 0, q0:m1], pr[:, 0, q0:m1],
                                    mask_t[:, 2 * mi, q0:m1])
                                nc.vector.tensor_mul(
                                    pr[:, 1, q0:m2], pr[:, 1, q0:m2],
                                    mask_t[:, 2 * mi + 1, q0:m2])
                            if p == 0:
                                nc.vector.tensor_add(acc[:], pr[:, 0, :],
                                                     pr[:, 1, :])
                            else:
                                ps = bsp.tile([128, TS], BF16, tag="ps", bufs=3)
                                nc.vector.tensor_add(ps[:, q0:], pr[:, 0, q0:],
                                                     pr[:, 1, q0:])
                                nc.vector.tensor_add(acc[:, q0:], acc[:, q0:],
                                                     ps[:, q0:])
                            if prev is not None:
                                pp, ppr = prev
                                for h2 in range(2):
                                    tkl = 2 * pp + h2 - 4 * ts
                                    a0 = 128 * tkl if tkl > 0 else 0
                                    nc.tensor.matmul(
                                        op[:, a0:], v_t[b][:, 2 * pp + h2, hh, :],
                                        ppr[:, h2, a0:],
                                        start=(pp == 0 and h2 == 0), stop=False)
                            prev = (p, pr)
                        pp, ppr = prev
                        for h2 in range(2):
                            tkl = 2 * pp + h2 - 4 * ts
                            a0 = 128 * tkl if tkl > 0 else 0
                            nc.tensor.matmul(
                                op[:, a0:], v_t[b][:, 2 * pp + h2, hh, :],
                                ppr[:, h2, a0:],
                                start=(pp == 0 and h2 == 0), stop=(h2 == 1))
                        pend = (op, acc, hh, ts)
                    # flush before the collective: it needs every tile's aos
                    epilogue(pend)
                    pend = None
                # ONE AllToAll per batch, carrying both heads: the gpsimd
                # queue blocks on each collective's completion, so per-head
                # collectives would stall the second head's epilogue DMAs
                # (and thus the next trigger) behind the first head's wait.
                nc.gpsimd.collective_compute(
                    "AllToAll", mybir.AluOpType.bypass,
                    replica_groups=[list(range(NC_))],
                    ins=[a2a_in.opt()], outs=[a2a_out.opt()])
                return a2a_out

            def load_aog(b, a2a_out, aogp):
                # prefetch a2a results into SBUF in 128KB chunks right after
                # the collectives are issued; runs under the following phase.
                aoG = [aogp.tile([128, NC_, ROWS], BF16, tag=f"aoG{b}{hh}",
                                 name=f"aoG{b}{hh}") for hh in range(HPC)]
                for hh in range(HPC):
                    src = a2a_out[:, hh, :, :].rearrange("s d q -> d s q")
                    for c in range(4):
                        nc.sync.dma_start(out=aoG[hh][:, 2 * c:2 * c + 2, :],
                                          in_=src[:, 2 * c:2 * c + 2, :])
                return aoG

            def outproj(b, aoG, yop, yps):
                # aoG[hh][d, src, q] == head (2*src+hh) for my ROWS of batch b
                # tile-at-a-time: each (tb, nb) chunk accumulates its 16
                # head contributions back-to-back, then evacuates while the
                # next chunk computes — output writes pipeline instead of
                # bursting at the end.
                for tb in range(2):
                    for nb in range(4):
                        yp = yps.tile([128, TS], F32, tag="yp", bufs=4)
                        for hh in range(HPC):
                            for s in range(NC_):
                                nc.tensor.matmul(
                                    yp[:], aoG[hh][:, s, tb * 128:(tb + 1) * 128],
                                    wo_t[:, 2 * s + hh, nb * TS:(nb + 1) * TS],
                                    start=(hh == 0 and s == 0),
                                    stop=(hh == 1 and s == NC_ - 1))
                        yo = yop.tile([128, TS], F32, tag="yo", bufs=4)
                        nc.vector.tensor_add(yo[:], yp[:],
                                             bo_t[:, nb * TS:(nb + 1) * TS])
                        # all output writes ride the scalar queue: the sync
                        # queue is parked on the next batch's a2a_out waits
                        # and would stall yo buffer recycling.
                        for c in range(4):
                            nc.scalar.dma_start(
                                out=out_d.ap()[b, tb * 128:(tb + 1) * 128,
                                               nb * TS + c * 128:
                                               nb * TS + (c + 1) * 128],
                                in_=yo[:, c * 128:(c + 1) * 128])

            # ---- phase 1: QKV projections for both batches (no collectives)
            with tc.tile_pool(name="s1w", bufs=1) as s1w, \
                    tc.tile_pool(name="xp", bufs=1) as xp, \
                    tc.tile_pool(name="qep", bufs=1) as qep, \
                    tc.tile_pool(name="tmp", bufs=1) as tmp, \
                    tc.tile_pool(name="s1ps", bufs=1, space="PSUM") as s1ps:
                wqk_t = [s1w.tile([128, 4 * 128], BF16, tag=f"wqk{ci}",
                                  name=f"wqk{ci}") for ci in range(NCH)]
                wv_t = [s1w.tile([128, 2 * 128], BF16, tag=f"wv{ci}",
                                 name=f"wv{ci}") for ci in range(NCH)]

                def new_xt():
                    return xp.tile([128, TS], BF16, tag="xt", name="xt", bufs=16)

                # head: the (b0, ts0) stage-in fans out over three queues so
                # the first matmul waits on exactly one weight chunk + one x
                # tile (256KB), not the whole stage-in.
                # ci0's weights + x go first (the first matmul's only deps),
                # then the small consts BEFORE the bulk weight stream: DMA
                # issue instructions block on ring credit, so anything queued
                # behind the x-prefetch flood lands tens of us late.
                xt0 = [new_xt()]
                nc.scalar.dma_start(out=wqk_t[0], in_=wqk_r[:, 0, :])
                nc.gpsimd.dma_start(out=wv_t[0], in_=wv_r[:, 0, :])
                nc.sync.dma_start(out=xt0[0], in_=xT_d.ap()[0, 0:128, 0:TS])
                load_consts()
                for ci in range(1, NCH):
                    nc.scalar.dma_start(out=wqk_t[ci], in_=wqk_r[:, ci, :])
                    nc.gpsimd.dma_start(out=wv_t[ci], in_=wv_r[:, ci, :])
                    xt = new_xt()
                    nc.sync.dma_start(
                        out=xt, in_=xT_d.ap()[0, ci * 128:(ci + 1) * 128, 0:TS])
                    xt0.append(xt)
                for b in range(B):
                    stage1(b, xt0, new_xt, qep, tmp, s1ps)

            # wo streams in while attention runs (DMA rings are idle there)
            for ci in range(NCH):
                nc.sync.dma_start(out=wo_t[:, ci, :], in_=wo_r[:, ci, :])

            # ---- phase 2: attention + exchanges --------------------------
            a2a_outs = {}
            aoGs = {}
            with tc.tile_pool(name="aogp", bufs=1) as aogp:
                with tc.tile_pool(name="atps", bufs=1, space="PSUM") as atps, \
                        tc.tile_pool(name="prp", bufs=1) as prp, \
                        tc.tile_pool(name="accp", bufs=1) as accp, \
                        tc.tile_pool(name="bsp", bufs=1) as bsp, \
                        tc.tile_pool(name="aosp", bufs=1) as aosp:
                    # pre-allocate PSUM tags in bank order; 'warm' lands on
                    # the spare 8th bank, hosting throwaway warm-up matmuls.
                    for _ in range(2):
                        atps.tile([128, 2, TS], F32, tag="st", bufs=2,
                                  name="st")
                        atps.tile([128, TS], F32, tag="op", bufs=2, name="op")
                    atps.tile([128, TS], F32, tag="sm", bufs=1, name="sm")
                    warm = atps.tile([64, 64], F32, tag="warm", bufs=1,
                                     name="warm")
                    # bridge the QKV->attention hand-off: dependency-free
                    # dummies sit at this program position in the PE stream
                    # (the scheduler is priority-ordered), so they fill the
                    # eviction-chain bubble and keep HAM at full clock.
                    for _ in range(90):
                        nc.tensor.matmul(warm[:], ones_b[:, 0:64],
                                         ones_b[:, 0:64],
                                         start=True, stop=True)
                    for b in range(B):
                        a2a_outs[b] = attention(b, atps, prp, accp, bsp, aosp)
                        aoGs[b] = load_aog(b, a2a_outs[b], aogp)
                    # bridge the attention->outproj hand-off the same way
                    for _ in range(40):
                        nc.tensor.matmul(warm[:], ones_b[:, 0:64],
                                         ones_b[:, 0:64],
                                         start=True, stop=True)

                # ---- phase 3: output projections -------------------------
                with tc.tile_pool(name="yps", bufs=1, space="PSUM") as yps, \
                        tc.tile_pool(name="yop", bufs=1) as yop:
                    for b in range(B):
                        outproj(b, aoGs[b], yop, yps)

    nc.compile()
    return nc


_NC_CACHE = None


def _get_program():
    global _NC_CACHE
    if _NC_CACHE is None:
        _NC_CACHE = _build_program()
    return _NC_CACHE


def make_in_maps(x, rope_cos, rope_sin, qkv_w, qkv_b, out_w, out_b):
    x = np.asarray(x, dtype=np.float32)
    qkv_w = np.asarray(qkv_w, dtype=np.float32)
    qkv_b = np.asarray(qkv_b, dtype=np.float32)
    out_w = np.asarray(out_w, dtype=np.float32)
    out_b = np.asarray(out_b, dtype=np.float32)

    xT = np.ascontiguousarray(x.transpose(0, 2, 1)).astype(BF16_NP)  # [B, D, T]
    cosT = np.ascontiguousarray(np.asarray(rope_cos, np.float32)[0, 0].T).astype(BF16_NP)
    sinTs = np.ascontiguousarray(np.asarray(rope_sin, np.float32)[0, 0].T).copy()
    sinTs[0:64, :] *= -1.0
    sinTs = sinTs.astype(BF16_NP)

    tk_idx = np.arange(128)[:, None]
    tq_idx = np.arange(TS)[None, :]
    masks = np.stack(
        [np.where(mi * 128 + tk_idx <= tq_idx, 1.0, 0.0) for mi in range(4)]
    ).astype(BF16_NP)                                           # [4, 128, TS]
    ones = np.ones((128, 128), BF16_NP)
    wo = np.ascontiguousarray(out_w.T).astype(BF16_NP)          # [D, D]
    bo = out_b.reshape(1, D)

    in_maps = []
    for c in range(NC_):
        h0 = HPC * c
        qr = qkv_w[h0 * 128:(h0 + HPC) * 128]                  # [256, D]
        kr = qkv_w[D + h0 * 128:D + (h0 + HPC) * 128]
        vr = qkv_w[2 * D + h0 * 128:2 * D + (h0 + HPC) * 128]
        wqk = np.ascontiguousarray(np.concatenate([qr, kr], 0).T).astype(BF16_NP)
        wv = np.ascontiguousarray(vr.T).astype(BF16_NP)        # [D, 256]
        bqk = np.stack(
            [qkv_b[h0 * 128:(h0 + 1) * 128],
             qkv_b[(h0 + 1) * 128:(h0 + 2) * 128],
             qkv_b[D + h0 * 128:D + (h0 + 1) * 128],
             qkv_b[D + (h0 + 1) * 128:D + (h0 + 2) * 128]], axis=1)  # [128, 4]
        bv = qkv_b[2 * D + h0 * 128:2 * D + (h0 + HPC) * 128].reshape(1, 256)
        in_maps.append({
            "xT": xT, "wqk": wqk, "wv": wv, "wo": wo,
            "cosT": cosT, "sinTs": sinTs, "masks": masks, "ones": ones,
            "bqk": np.ascontiguousarray(bqk),
            "bv": np.ascontiguousarray(bv).astype(BF16_NP),
            "bo": bo,
        })
    return in_maps


def assemble(results):
    y = np.empty((B, T, D), dtype=np.float32)
    for c in range(NC_):
        y[:, c * ROWS:(c + 1) * ROWS, :] = results[c]["out"]
    return y


def run(inputs, trace=False, trace_cores=None):
    nc = _get_program()
    in_maps = make_in_maps(**inputs)
    res = run_bass_kernel_spmd(
        nc, in_maps, list(range(NC_)), trace=trace,
        trace_cores=trace_cores if trace else None)
    return assemble(res.results), res


def kernel(**inputs) -> np.ndarray:
    y, _ = run(inputs, trace=False)
    return y
